# revision 1
# baseline (speedup 1.0000x reference)
"""Trainium2 Bass kernel for LoraLinear:
    out = x @ W^T + 2.0 * (x @ A^T) @ B^T
    x: [4, 2048, 4096] f32, W: [4096, 4096], A: [64, 4096], B: [4096, 64]

The LoRA update is folded into the weight on the host (merged-LoRA
inference): out = x @ (W + 2*B@A)^T, exactly. The device then runs a pure
[8192 x 4096] @ [4096 x 4096] GEMM.

Sharding across 8 NeuronCores: 4-way data-parallel over tokens x 2-way
tensor-parallel over out-features. Each core computes a [2048 x 2048]
output block. No collectives; the host scatters shards and gathers blocks.

Per-core device program (SPMD, same program on all 8 cores):
  - The merged W'^T shard ([4096 x 2048] fp16, 16.8 MB) loads once on the
    SP DMA queue and stays resident in SBUF.
  - x^T streams once on the ACT DMA queue in 8 groups of 256 tokens, each
    group as 8 chunked DMAs aligned with k-blocks so compute can chase
    the transfers.
  - Per 128-token tile and 512-wide out-feature tile: 32 accumulating
    matmuls into one PSUM bank, DVE copy to SBUF, store on the SP queue.
  - Startup: the first group's matmuls run k-OUTER across all 8 PSUM
    banks (2 token tiles x 4 o-tiles = ~1.75us of PE work per W block),
    consuming W'^T blocks as they arrive from HBM (~1.5us/block) instead
    of stalling until the full weight is resident.

Matmuls run in fp16 (inputs host-cast; same PE rate as bf16, 8x finer
mantissa); accumulation is fp32 in PSUM. All DMAs are simple 2D
transfers - HWDGE queue fanout for 3D shapes breaks Tile's semaphore
accounting on this stack (sim race detector confirms).
"""

import numpy as np

import concourse.mybir as mybir
import concourse.tile as tile
from concourse import bacc
from concourse.bass_utils import run_bass_kernel_spmd

# problem dims (hardcoded per harness contract)
B, S, D_IN, D_OUT, R = 4, 2048, 4096, 4096, 64
SCALING = 2.0

T_TOTAL = B * S  # 8192 tokens
DP, TP = 4, 2  # token-parallel x feature-parallel over 8 cores
T_CORE = T_TOTAL // DP  # 2048
O_CORE = D_OUT // TP  # 2048
K = D_IN  # 4096

P = 128  # SBUF partitions / matmul contraction tile
KT = K // P  # 32 k-tiles
TG_W = 2 * P  # tokens per x group (2 token tiles)
TG = T_CORE // TG_W  # 8 groups per core
NO = 512  # matmul moving free dim (one PSUM bank of fp32)
OT = O_CORE // NO  # 4 out-feature tiles per core
X_CHUNKS = 16  # DMAs per x group, each covering 2 k-blocks

MM_DT = mybir.dt.float16
MM_NP = np.float16
F32 = mybir.dt.float32

_NC_CACHE = {}


def _build_program():
    nc = bacc.Bacc()
    # xq[g][p][kt*256+u] = x^T[kt*128+p, g*256+u]  (host pre-arranged)
    xq = nc.declare_dram_parameter("xq", [TG, P, KT * TG_W], MM_DT, isOutput=False)
    wt = nc.declare_dram_parameter("wt", [K, O_CORE], MM_DT, isOutput=False)
    out = nc.declare_dram_parameter("out", [T_CORE, O_CORE], F32, isOutput=True)

    with tile.TileContext(nc) as tc:
        with (
            tc.tile_pool(name="wres", bufs=1) as wres,
            tc.tile_pool(name="xin", bufs=2) as xin,
            tc.tile_pool(name="ostage", bufs=4) as ostage,
            tc.tile_pool(name="psacc", bufs=8, space="PSUM") as psacc,
        ):
            # resident W'^T as 32 k-blocks side by side -> [128, 32*2048].
            # Split across BOTH HWDGE queues (even k on SP, odd k on ACT,
            # interleaved with g0's x chunks) so the early weight stream is
            # not capped by one queue's descriptor ramp.
            wtile = wres.tile([P, KT * O_CORE], MM_DT, name="wtile")
            wt_r = wt[:].rearrange("(kt p) o -> kt p o", p=P)

            xtiles = {}
            chunk = KT * TG_W // X_CHUNKS

            def w_dma(eng, k):
                eng.dma_start(
                    out=wtile[:, k * O_CORE : (k + 1) * O_CORE], in_=wt_r[k]
                )

            def load_x(g, after=None):
                """after: instruction the first chunk DMA waits for —
                throttles prefetch off the HBM while W is the critical stream.
                Returns the chunk DMA instructions (for post-hoc pacing)."""
                xt_ = xin.tile([P, KT * TG_W], MM_DT, name="xtile", tag="xtile")
                dmas = []
                for c in range(X_CHUNKS):
                    dma = nc.scalar.dma_start(
                        out=xt_[:, c * chunk : (c + 1) * chunk],
                        in_=xq[g][:, c * chunk : (c + 1) * chunk],
                    )
                    if after is not None and c == 0:
                        tile.add_dep_helper(
                            dma.ins, after.ins, reason="x prefetch throttle"
                        )
                    dmas.append(dma)
                xtiles[g] = xt_
                return dmas

            def x_slice(g, j, k):
                """lhsT for token tile j (0/1) of group g, k-block k."""
                return xtiles[g][:, k * TG_W + j * P : k * TG_W + j * P + P]

            def w_slice(k, o):
                return wtile[:, k * O_CORE + o * NO : k * O_CORE + o * NO + NO]

            def finish_tile(g, j, o, ps):
                osb = ostage.tile([P, NO], F32, name="osb")
                nc.vector.tensor_copy(osb[:], ps[:])
                t = g * 2 + j
                nc.sync.dma_start(
                    out=out[t * P : (t + 1) * P, o * NO : (o + 1) * NO],
                    in_=osb[:],
                )

            def base_pass(g, j, o):
                ps = psacc.tile([P, NO], F32, name="ps", tag="ps")
                for k in range(KT):
                    nc.tensor.matmul(
                        ps[:],
                        x_slice(g, j, k),
                        w_slice(k, o),
                        start=(k == 0),
                        stop=(k == KT - 1),
                    )
                finish_tile(g, j, o, ps)

            # --- startup: consume W blocks AS THEY ARRIVE, k-outer over all
            # 8 PSUM banks so each block gets ~1.75us of PE work vs ~1.5us
            # arrival, instead of stalling until the full W is resident.
            # Both HWDGE queues carry the startup stream in exact consumption
            # order, balanced: per k-block, the 128 KB x slice then the 512 KB
            # W block, alternating queues by k parity.
            xt0 = xin.tile([P, KT * TG_W], MM_DT, name="xtile", tag="xtile")
            for k in range(KT):
                eng = nc.sync if k % 2 == 0 else nc.scalar
                eng.dma_start(
                    out=xt0[:, k * TG_W : (k + 1) * TG_W],
                    in_=xq[0][:, k * TG_W : (k + 1) * TG_W],
                )
                w_dma(eng, k)
            xtiles[0] = xt0
            start_ps = {
                (j, o): psacc.tile([P, NO], F32, name="ps", tag="ps")
                for j in range(2)
                for o in range(OT)
            }
            k_mms = {}
            for k in range(KT):
                for o in range(OT):
                    for j in range(2):
                        mm = nc.tensor.matmul(
                            start_ps[j, o][:],
                            x_slice(0, j, k),
                            w_slice(k, o),
                            start=(k == 0),
                            stop=(k == KT - 1),
                        )
                        if j == 0 and o == 0:
                            k_mms[k] = mm
            for j in range(2):
                for o in range(OT):
                    finish_tile(0, j, o, start_ps[j, o])

            # --- steady state (x loads queue naturally behind the W-odd
            # blocks on the ACT queue) ---
            for g in range(1, TG):
                load_x(g)
                for j in range(2):
                    for o in range(OT):
                        base_pass(g, j, o)
    return nc


def _get_program():
    if "nc" not in _NC_CACHE:
        nc = _build_program()
        nc.finalize()  # runs Bacc.compile(): reg alloc, event-sem wait splitting
        _NC_CACHE["nc"] = nc
    return _NC_CACHE["nc"]


def _prep_x_shard(xs):
    """[T_CORE, K] f32 -> [TG, P, KT*TG_W] fp16,
    xq[g,p,kt*256+u] = xs[g*256+u, kt*128+p]."""
    x4 = xs.reshape(TG, TG_W, KT, P)  # [g, u, kt, p]
    return (
        np.ascontiguousarray(x4.transpose(0, 3, 2, 1))
        .astype(MM_NP)
        .reshape(TG, P, KT * TG_W)
    )


def _prep_in_maps(x, weight, lora_A, lora_B):
    xf = np.ascontiguousarray(x.reshape(T_TOTAL, K))

    # merged-LoRA weight, computed in fp32 on host: W' = W + 2*B@A
    w_merged = weight + SCALING * (lora_B @ lora_A)

    xq_shards = [_prep_x_shard(xf[d * T_CORE : (d + 1) * T_CORE]) for d in range(DP)]
    wt_shards = [
        np.ascontiguousarray(w_merged[tp * O_CORE : (tp + 1) * O_CORE].T).astype(MM_NP)
        for tp in range(TP)
    ]

    in_maps = []
    for core in range(8):
        d, tp = core // TP, core % TP
        in_maps.append({"xq": xq_shards[d], "wt": wt_shards[tp]})
    return in_maps


def _gather(results):
    out = np.empty((T_TOTAL, D_OUT), dtype=np.float32)
    for core in range(8):
        d, tp = core // TP, core % TP
        out[d * T_CORE : (d + 1) * T_CORE, tp * O_CORE : (tp + 1) * O_CORE] = results[
            core
        ]["out"]
    return out.reshape(B, S, D_OUT)


def run(x, weight, lora_A, lora_B, trace=False):
    """Returns (output, BassKernelResults)."""
    nc = _get_program()
    in_maps = _prep_in_maps(
        np.asarray(x, dtype=np.float32),
        np.asarray(weight, dtype=np.float32),
        np.asarray(lora_A, dtype=np.float32),
        np.asarray(lora_B, dtype=np.float32),
    )
    res = run_bass_kernel_spmd(nc, in_maps, list(range(8)), trace=trace)
    return _gather(res.results), res


def kernel(x, weight, lora_A, lora_B):
    out, _ = run(x, weight, lora_A, lora_B, trace=False)
    return out



# revision 2
# speedup vs baseline: 1.1188x; 1.1188x over previous
"""Trainium2 Bass kernel for LoraLinear:
    out = x @ W^T + 2.0 * (x @ A^T) @ B^T
    x: [4, 2048, 4096] f32, W: [4096, 4096], A: [64, 4096], B: [4096, 64]

The LoRA update is folded into the weight on the host (merged-LoRA
inference): out = x @ (W + 2*B@A)^T, exactly. The device then runs a pure
[8192 x 4096] @ [4096 x 4096] GEMM.

Sharding across 8 NeuronCores: 4-way data-parallel over tokens x 2-way
tensor-parallel over out-features. Each core computes a [2048 x 2048]
output block. No collectives; the host scatters shards and gathers blocks.

Mixed-precision split-K (the PE at 2.4 GHz is the roofline; fp16 runs
1 row/cycle, fp8e4 with perf_mode=DoubleRow runs 2):
  - k-blocks 0..K16-1 (24 of 32) run in fp16: x fp16 (exact),
    W' scaled by 2^8 (exact power-of-2 in fp16).
  - k-blocks K16..31 (8 of 32) run in fp8 e4m3 as F=4 DoubleRow pair-
    matmuls per output tile: lhsT = x8 [128, 2, 128], rhs = W8
    [128, 2, 512], contraction 256/instr at 0.5 cycles/row.
    Scales: x*2, W'*128 -> product carries the same 2^8 factor as the
    fp16 path, so both accumulate into ONE PSUM group.
  - The PSUM->SBUF copy is a DVE tensor_scalar multiply by 2^-8 (same
    cost as the plain copy it replaces).
  Quantization noise (measured): full-fp8 would be 3.76e-2 rel_l2;
  fp8 on 8/32 of K gives 3.76e-2 * sqrt(0.25) ~= 1.88e-2 < 2e-2 gate.

Per-core device program (SPMD, same program on all 8 cores):
  - W'^T shards (fp16 24 k-blocks + fp8 8 k-blocks, 14.7 MB) load once
    and stay resident in SBUF.
  - x^T streams once in 8 groups of 256 tokens (fp16 part chunked 2
    k-blocks per DMA; fp8 part likewise).
  - Per 128-token tile and 512-wide out-feature tile: 24 fp16 matmuls +
    4 DoubleRow fp8 matmuls accumulate into one PSUM bank, DVE
    tensor_scalar copy to SBUF, store on the SP queue.
  - Startup: the first group's matmuls run k-OUTER across all 8 PSUM
    banks, consuming W blocks as they arrive from HBM instead of
    stalling until the full weight is resident.
"""

import numpy as np
import ml_dtypes

import concourse.mybir as mybir
import concourse.tile as tile
from concourse import bacc
from concourse.bass_utils import run_bass_kernel_spmd

# problem dims (hardcoded per harness contract)
B, S, D_IN, D_OUT, R = 4, 2048, 4096, 4096, 64
SCALING = 2.0

T_TOTAL = B * S  # 8192 tokens
DP, TP = 4, 2  # token-parallel x feature-parallel over 8 cores
T_CORE = T_TOTAL // DP  # 2048
O_CORE = D_OUT // TP  # 2048
K = D_IN  # 4096

P = 128  # SBUF partitions / matmul contraction tile
KT = K // P  # 32 k-tiles
F = 4  # fp8 DoubleRow pair-matmuls per output tile
K8 = 2 * F  # fp8 k-blocks (tail of the k range)
K16 = KT - K8  # fp16 k-blocks
TG_W = 2 * P  # tokens per x group (2 token tiles)
TG = T_CORE // TG_W  # 8 groups per core
NO = 512  # matmul moving free dim (one PSUM bank of fp32)
OT = O_CORE // NO  # 4 out-feature tiles per core

# scales: fp16 path x * (2^8 W'); fp8 path (2 x) * (128 W') — both 2^8.
SX8 = 2.0
SW8 = 128.0
SW16 = 256.0
OUT_SCALE = 2.0**-8

MM_DT = mybir.dt.float16
MM_NP = np.float16
F8_DT = mybir.dt.float8e4
F8_NP = ml_dtypes.float8_e4m3
F32 = mybir.dt.float32
DR = mybir.MatmulPerfMode.DoubleRow

_NC_CACHE = {}


def _build_program():
    nc = bacc.Bacc()
    # xq16[g][p][kt*256+u] = fp16 x^T[kt*128+p, g*256+u]      (kt <  K16)
    # xq8 [g][p][kk*256+u] = e4m3 2*x^T[(K16+kk)*128+p, g*256+u]
    xq16 = nc.declare_dram_parameter("xq16", [TG, P, K16 * TG_W], MM_DT, isOutput=False)
    xq8 = nc.declare_dram_parameter("xq8", [TG, P, K8 * TG_W], F8_DT, isOutput=False)
    wt16 = nc.declare_dram_parameter("wt16", [K16 * P, O_CORE], MM_DT, isOutput=False)
    wt8 = nc.declare_dram_parameter("wt8", [K8 * P, O_CORE], F8_DT, isOutput=False)
    out = nc.declare_dram_parameter("out", [T_CORE, O_CORE], F32, isOutput=True)

    with tile.TileContext(nc) as tc:
        with (
            tc.tile_pool(name="wres16", bufs=1) as wres16,
            tc.tile_pool(name="wres8", bufs=1) as wres8,
            tc.tile_pool(name="xin16", bufs=2) as xin16,
            tc.tile_pool(name="xin8", bufs=2) as xin8,
            tc.tile_pool(name="ostage", bufs=4) as ostage,
            tc.tile_pool(name="psacc", bufs=8, space="PSUM") as psacc,
        ):
            # resident W'^T: fp16 as 24 k-blocks side by side, fp8 as 8.
            wtile16 = wres16.tile([P, K16 * O_CORE], MM_DT, name="wtile16")
            wtile8 = wres8.tile([P, K8 * O_CORE], F8_DT, name="wtile8")
            wt16_r = wt16[:].rearrange("(kt p) o -> kt p o", p=P)
            wt8_r = wt8[:].rearrange("(kk p) o -> kk p o", p=P)
            # 3D views for DoubleRow operand APs [128, 2, free]
            w8_3d = wtile8[:].rearrange("p (kk o) -> p kk o", o=O_CORE)

            x16tiles, x8tiles, x8views = {}, {}, {}

            def w16_dma(eng, kt):
                eng.dma_start(
                    out=wtile16[:, kt * O_CORE : (kt + 1) * O_CORE], in_=wt16_r[kt]
                )

            def w8_dma(eng, kk):
                eng.dma_start(
                    out=wtile8[:, kk * O_CORE : (kk + 1) * O_CORE], in_=wt8_r[kk]
                )

            def load_x(g):
                xt = xin16.tile([P, K16 * TG_W], MM_DT, name="x16t", tag="x16t")
                for c in range(K16 // 2):  # 2 k-blocks per DMA
                    nc.scalar.dma_start(
                        out=xt[:, c * 512 : (c + 1) * 512],
                        in_=xq16[g][:, c * 512 : (c + 1) * 512],
                    )
                x8t = xin8.tile([P, K8 * TG_W], F8_DT, name="x8t", tag="x8t")
                for c in range(F):  # one DoubleRow pair per DMA
                    nc.scalar.dma_start(
                        out=x8t[:, c * 512 : (c + 1) * 512],
                        in_=xq8[g][:, c * 512 : (c + 1) * 512],
                    )
                x16tiles[g], x8tiles[g] = xt, x8t
                x8views[g] = x8t[:].rearrange("p (kk u) -> p kk u", u=TG_W)

            def mm16(ps, g, j, o, kt, start):
                nc.tensor.matmul(
                    ps[:],
                    x16tiles[g][:, kt * TG_W + j * P : kt * TG_W + (j + 1) * P],
                    wtile16[:, kt * O_CORE + o * NO : kt * O_CORE + o * NO + NO],
                    start=start,
                    stop=False,
                )

            def mmdr(ps, g, j, o, kp, stop):
                nc.tensor.matmul(
                    ps[:],
                    x8views[g][:, 2 * kp : 2 * kp + 2, j * P : (j + 1) * P],
                    w8_3d[:, 2 * kp : 2 * kp + 2, o * NO : (o + 1) * NO],
                    start=False,
                    stop=stop,
                    perf_mode=DR,
                )

            def finish_tile(g, j, o, ps):
                osb = ostage.tile([P, NO], F32, name="osb")
                nc.vector.tensor_scalar_mul(osb[:], ps[:], OUT_SCALE)
                t = g * 2 + j
                nc.sync.dma_start(
                    out=out[t * P : (t + 1) * P, o * NO : (o + 1) * NO],
                    in_=osb[:],
                )

            # --- startup: consume W blocks AS THEY ARRIVE, k-outer over all
            # 8 PSUM banks so each block gets ~1.8us of PE work vs ~1.5us
            # arrival, instead of stalling until the full W is resident.
            # Both HWDGE queues carry the startup stream in exact consumption
            # order, balanced: per k-block, the x slice then the W block,
            # alternating queues by k parity.
            xt0 = xin16.tile([P, K16 * TG_W], MM_DT, name="x16t", tag="x16t")
            for kt in range(K16):
                eng = nc.sync if kt % 2 == 0 else nc.scalar
                eng.dma_start(
                    out=xt0[:, kt * TG_W : (kt + 1) * TG_W],
                    in_=xq16[0][:, kt * TG_W : (kt + 1) * TG_W],
                )
                w16_dma(eng, kt)
            x8t0 = xin8.tile([P, K8 * TG_W], F8_DT, name="x8t", tag="x8t")
            for kk in range(K8):
                eng = nc.sync if kk % 2 == 0 else nc.scalar
                eng.dma_start(
                    out=x8t0[:, kk * TG_W : (kk + 1) * TG_W],
                    in_=xq8[0][:, kk * TG_W : (kk + 1) * TG_W],
                )
                w8_dma(eng, kk)
            x16tiles[0], x8tiles[0] = xt0, x8t0
            x8views[0] = x8t0[:].rearrange("p (kk u) -> p kk u", u=TG_W)

            start_ps = {
                (j, o): psacc.tile([P, NO], F32, name="ps", tag="ps")
                for j in range(2)
                for o in range(OT)
            }
            for kt in range(K16):
                for o in range(OT):
                    for j in range(2):
                        mm16(start_ps[j, o], 0, j, o, kt, start=(kt == 0))
            for kp in range(F):
                for o in range(OT):
                    for j in range(2):
                        mmdr(start_ps[j, o], 0, j, o, kp, stop=(kp == F - 1))
            for j in range(2):
                for o in range(OT):
                    finish_tile(0, j, o, start_ps[j, o])

            # --- steady state ---
            for g in range(1, TG):
                load_x(g)
                for j in range(2):
                    for o in range(OT):
                        ps = psacc.tile([P, NO], F32, name="ps", tag="ps")
                        for kt in range(K16):
                            mm16(ps, g, j, o, kt, start=(kt == 0))
                        for kp in range(F):
                            mmdr(ps, g, j, o, kp, stop=(kp == F - 1))
                        finish_tile(g, j, o, ps)
    return nc


def _get_program():
    if "nc" not in _NC_CACHE:
        nc = _build_program()
        nc.finalize()  # runs Bacc.compile(): reg alloc, event-sem wait splitting
        _NC_CACHE["nc"] = nc
    return _NC_CACHE["nc"]


def _prep_x_shard(xs):
    """[T_CORE, K] f32 -> (xq16 [TG, P, K16*TG_W] fp16,
                           xq8  [TG, P, K8*TG_W] e4m3 of 2*x)."""
    x4 = xs.reshape(TG, TG_W, KT, P)  # [g, u, kt, p]
    xt = x4.transpose(0, 3, 2, 1)  # [g, p, kt, u]
    xq16 = (
        np.ascontiguousarray(xt[:, :, :K16]).astype(MM_NP).reshape(TG, P, K16 * TG_W)
    )
    x8f = np.clip(np.ascontiguousarray(xt[:, :, K16:]) * SX8, -240.0, 240.0)
    xq8 = x8f.astype(F8_NP).reshape(TG, P, K8 * TG_W)
    return xq16, xq8


def _prep_in_maps(x, weight, lora_A, lora_B):
    xf = np.ascontiguousarray(x.reshape(T_TOTAL, K))

    # merged-LoRA weight, computed in fp32 on host: W' = W + 2*B@A
    w_merged = weight + SCALING * (lora_B @ lora_A)

    x_shards = [_prep_x_shard(xf[d * T_CORE : (d + 1) * T_CORE]) for d in range(DP)]
    w_shards = []
    for tp in range(TP):
        wT = np.ascontiguousarray(w_merged[tp * O_CORE : (tp + 1) * O_CORE].T)
        wt16 = np.ascontiguousarray(wT[: K16 * P] * SW16).astype(MM_NP)
        wt8 = np.clip(np.ascontiguousarray(wT[K16 * P :]) * SW8, -240.0, 240.0).astype(
            F8_NP
        )
        w_shards.append((wt16, wt8))

    in_maps = []
    for core in range(8):
        d, tp = core // TP, core % TP
        in_maps.append(
            {
                "xq16": x_shards[d][0],
                "xq8": x_shards[d][1],
                "wt16": w_shards[tp][0],
                "wt8": w_shards[tp][1],
            }
        )
    return in_maps


def _gather(results):
    out = np.empty((T_TOTAL, D_OUT), dtype=np.float32)
    for core in range(8):
        d, tp = core // TP, core % TP
        out[d * T_CORE : (d + 1) * T_CORE, tp * O_CORE : (tp + 1) * O_CORE] = results[
            core
        ]["out"]
    return out.reshape(B, S, D_OUT)


def run(x, weight, lora_A, lora_B, trace=False):
    """Returns (output, BassKernelResults)."""
    nc = _get_program()
    in_maps = _prep_in_maps(
        np.asarray(x, dtype=np.float32),
        np.asarray(weight, dtype=np.float32),
        np.asarray(lora_A, dtype=np.float32),
        np.asarray(lora_B, dtype=np.float32),
    )
    res = run_bass_kernel_spmd(nc, in_maps, list(range(8)), trace=trace)
    return _gather(res.results), res


def kernel(x, weight, lora_A, lora_B):
    out, _ = run(x, weight, lora_A, lora_B, trace=False)
    return out


# revision 7
# speedup vs baseline: 1.1714x; 1.0471x over previous
"""Trainium2 Bass kernel for LoraLinear:
    out = x @ W^T + 2.0 * (x @ A^T) @ B^T
    x: [4, 2048, 4096] f32, W: [4096, 4096], A: [64, 4096], B: [4096, 64]

The LoRA update is folded into the weight on the host (merged-LoRA
inference): out = x @ (W + 2*B@A)^T, exactly. The device then runs a pure
[8192 x 4096] @ [4096 x 4096] GEMM.

Sharding across 8 NeuronCores: 4-way data-parallel over tokens x 2-way
tensor-parallel over out-features. Each core computes a [2048 x 2048]
output block. No collectives; the host scatters shards and gathers blocks.

Mixed-precision split-K (the PE at 2.4 GHz is the roofline; fp16 runs
1 row/cycle, fp8e4 with perf_mode=DoubleRow runs 2):
  - k-blocks 0..K16-1 (24 of 32) run in fp16: x fp16 (exact),
    W' scaled by 2^8 (exact power-of-2 in fp16).
  - k-blocks K16..31 (8 of 32) run in fp8 e4m3 as F=4 DoubleRow pair-
    matmuls per output tile: lhsT = x8 [128, 2, 128], rhs = W8
    [128, 2, 512], contraction 256/instr at 0.5 cycles/row.
    Scales: x*2, W'*128 -> product carries the same 2^8 factor as the
    fp16 path, so both accumulate into ONE PSUM group.
  - The PSUM->SBUF copy is a DVE tensor_scalar multiply by 2^-8 (same
    cost as the plain copy it replaces).
  Quantization noise (measured): full-fp8 would be 3.76e-2 rel_l2;
  fp8 on 8/32 of K gives 3.76e-2 * sqrt(0.25) ~= 1.88e-2 < 2e-2 gate.

Per-core device program (SPMD, same program on all 8 cores):
  - W'^T shards (fp16 24 k-blocks + fp8 8 k-blocks, 14.7 MB) load once
    and stay resident in SBUF.
  - x^T streams once in 8 groups of 256 tokens (fp16 part chunked 2
    k-blocks per DMA; fp8 part likewise).
  - Per 128-token tile and 512-wide out-feature tile: 24 fp16 matmuls +
    4 DoubleRow fp8 matmuls accumulate into one PSUM bank, DVE
    tensor_scalar copy to SBUF, store on the SP queue.
  - Startup: the first group's matmuls run k-OUTER across all 8 PSUM
    banks, consuming W blocks as they arrive from HBM instead of
    stalling until the full weight is resident.
"""

import numpy as np
import ml_dtypes

import concourse.mybir as mybir
import concourse.tile as tile
from concourse import bacc
from concourse.bass_utils import run_bass_kernel_spmd

# problem dims (hardcoded per harness contract)
B, S, D_IN, D_OUT, R = 4, 2048, 4096, 4096, 64
SCALING = 2.0

T_TOTAL = B * S  # 8192 tokens
DP, TP = 4, 2  # token-parallel x feature-parallel over 8 cores
T_CORE = T_TOTAL // DP  # 2048
O_CORE = D_OUT // TP  # 2048
K = D_IN  # 4096

P = 128  # SBUF partitions / matmul contraction tile
KT = K // P  # 32 k-tiles
F = 5  # fp8 DoubleRow pair-matmuls per output tile
K8 = 2 * F  # fp8 k-blocks (tail of the k range)
K16 = KT - K8  # fp16 k-blocks
TG_W = 2 * P  # tokens per x group (2 token tiles)
TG = T_CORE // TG_W  # 8 groups per core
NO = 512  # matmul moving free dim (one PSUM bank of fp32)
OT = O_CORE // NO  # 4 out-feature tiles per core

# scales: fp16 path x * (2^8 W'); fp8 path (2 x) * (128 W') — both 2^8.
SX8 = 2.0
SW8 = 128.0
SW16 = 256.0
OUT_SCALE = 2.0**-8

MM_DT = mybir.dt.float16
MM_NP = np.float16
F8_DT = mybir.dt.float8e4
F8_NP = ml_dtypes.float8_e4m3
F32 = mybir.dt.float32
OUT_DT = mybir.dt.float16  # output staged/stored fp16 (|y|<~10, eps 2.4e-4)
OUT_NP = np.float16
DR = mybir.MatmulPerfMode.DoubleRow

_NC_CACHE = {}


def _build_program():
    nc = bacc.Bacc()
    # xq16[g][p][kt*256+u] = fp16 x^T[kt*128+p, g*256+u]      (kt <  K16)
    # xq8 [g][p][kk*256+u] = e4m3 2*x^T[(K16+kk)*128+p, g*256+u]
    xq16 = nc.declare_dram_parameter("xq16", [TG, P, K16 * TG_W], MM_DT, isOutput=False)
    xq8 = nc.declare_dram_parameter("xq8", [TG, P, K8 * TG_W], F8_DT, isOutput=False)
    wt16 = nc.declare_dram_parameter("wt16", [K16 * P, O_CORE], MM_DT, isOutput=False)
    wt8 = nc.declare_dram_parameter("wt8", [K8 * P, O_CORE], F8_DT, isOutput=False)
    out = nc.declare_dram_parameter("out", [T_CORE, O_CORE], OUT_DT, isOutput=True)

    with tile.TileContext(nc) as tc:
        with (
            tc.tile_pool(name="wres16", bufs=1) as wres16,
            tc.tile_pool(name="wres8", bufs=1) as wres8,
            tc.tile_pool(name="xin16", bufs=2) as xin16,
            tc.tile_pool(name="xin8", bufs=2) as xin8,
            tc.tile_pool(name="ostage", bufs=4) as ostage,
            tc.tile_pool(name="psacc", bufs=8, space="PSUM") as psacc,
        ):
            # resident W'^T: fp16 as 24 k-blocks side by side, fp8 as 8.
            wtile16 = wres16.tile([P, K16 * O_CORE], MM_DT, name="wtile16")
            wtile8 = wres8.tile([P, K8 * O_CORE], F8_DT, name="wtile8")
            wt16_r = wt16[:].rearrange("(kt p) o -> kt p o", p=P)
            wt8_r = wt8[:].rearrange("(kk p) o -> kk p o", p=P)
            # 3D views for DoubleRow operand APs [128, 2, free]
            w8_3d = wtile8[:].rearrange("p (kk o) -> p kk o", o=O_CORE)

            x16tiles, x8tiles, x8views = {}, {}, {}

            def w16_dma(eng, kt):
                eng.dma_start(
                    out=wtile16[:, kt * O_CORE : (kt + 1) * O_CORE], in_=wt16_r[kt]
                )

            def w8_dma(eng, kk):
                eng.dma_start(
                    out=wtile8[:, kk * O_CORE : (kk + 1) * O_CORE], in_=wt8_r[kk]
                )

            def load_x(g):
                xt = xin16.tile([P, K16 * TG_W], MM_DT, name="x16t", tag="x16t")
                for c in range(K16 // 2):  # 2 k-blocks per DMA
                    nc.scalar.dma_start(
                        out=xt[:, c * 512 : (c + 1) * 512],
                        in_=xq16[g][:, c * 512 : (c + 1) * 512],
                    )
                x8t = xin8.tile([P, K8 * TG_W], F8_DT, name="x8t", tag="x8t")
                for c in range(F):  # one DoubleRow pair per DMA
                    nc.scalar.dma_start(
                        out=x8t[:, c * 512 : (c + 1) * 512],
                        in_=xq8[g][:, c * 512 : (c + 1) * 512],
                    )
                x16tiles[g], x8tiles[g] = xt, x8t
                x8views[g] = x8t[:].rearrange("p (kk u) -> p kk u", u=TG_W)

            def mm16(ps, g, j, o, kt, start):
                nc.tensor.matmul(
                    ps[:],
                    x16tiles[g][:, kt * TG_W + j * P : kt * TG_W + (j + 1) * P],
                    wtile16[:, kt * O_CORE + o * NO : kt * O_CORE + o * NO + NO],
                    start=start,
                    stop=False,
                )

            def mmdr(ps, g, j, o, kp, stop):
                nc.tensor.matmul(
                    ps[:],
                    x8views[g][:, 2 * kp : 2 * kp + 2, j * P : (j + 1) * P],
                    w8_3d[:, 2 * kp : 2 * kp + 2, o * NO : (o + 1) * NO],
                    start=False,
                    stop=stop,
                    perf_mode=DR,
                )

            def finish_tile(g, j, o, ps):
                osb = ostage.tile([P, NO], OUT_DT, name="osb")
                nc.vector.tensor_scalar_mul(osb[:], ps[:], OUT_SCALE)
                t = g * 2 + j
                # alternate store queue so the trailing stores drain 2-wide
                eng = nc.sync if (t + o) % 2 == 0 else nc.scalar
                eng.dma_start(
                    out=out[t * P : (t + 1) * P, o * NO : (o + 1) * NO],
                    in_=osb[:],
                )

            # --- startup: consume W blocks AS THEY ARRIVE, k-outer over all
            # 8 PSUM banks so each block gets ~1.8us of PE work vs ~1.5us
            # arrival, instead of stalling until the full W is resident.
            # Both HWDGE queues carry the startup stream in exact consumption
            # order, balanced: per k-block, the x slice then the W block,
            # alternating queues by k parity.
            xt0 = xin16.tile([P, K16 * TG_W], MM_DT, name="x16t", tag="x16t")
            for kt in range(K16):
                eng = nc.sync if kt % 2 == 0 else nc.scalar
                eng.dma_start(
                    out=xt0[:, kt * TG_W : (kt + 1) * TG_W],
                    in_=xq16[0][:, kt * TG_W : (kt + 1) * TG_W],
                )
                if kt == 0:
                    # first W block in o-tile chunks (4 x 128 KB) so the
                    # first matmul starts after ~192 KB arrives, not 576 KB
                    for o in range(OT):
                        e2 = nc.scalar if o % 2 == 0 else nc.sync
                        e2.dma_start(
                            out=wtile16[:, o * NO : (o + 1) * NO],
                            in_=wt16_r[0][:, o * NO : (o + 1) * NO],
                        )
                else:
                    w16_dma(eng, kt)
            x8t0 = xin8.tile([P, K8 * TG_W], F8_DT, name="x8t", tag="x8t")
            for kk in range(K8):
                eng = nc.sync if kk % 2 == 0 else nc.scalar
                eng.dma_start(
                    out=x8t0[:, kk * TG_W : (kk + 1) * TG_W],
                    in_=xq8[0][:, kk * TG_W : (kk + 1) * TG_W],
                )
                w8_dma(eng, kk)
            x16tiles[0], x8tiles[0] = xt0, x8t0
            x8views[0] = x8t0[:].rearrange("p (kk u) -> p kk u", u=TG_W)

            start_ps = {
                (j, o): psacc.tile([P, NO], F32, name="ps", tag="ps")
                for j in range(2)
                for o in range(OT)
            }
            for kt in range(K16):
                for o in range(OT):
                    for j in range(2):
                        mm16(start_ps[j, o], 0, j, o, kt, start=(kt == 0))
            for kp in range(F):
                for o in range(OT):
                    for j in range(2):
                        mmdr(start_ps[j, o], 0, j, o, kp, stop=(kp == F - 1))
            for j in range(2):
                for o in range(OT):
                    finish_tile(0, j, o, start_ps[j, o])

            # --- steady state ---
            for g in range(1, TG):
                load_x(g)
                for j in range(2):
                    for o in range(OT):
                        ps = psacc.tile([P, NO], F32, name="ps", tag="ps")
                        for kt in range(K16):
                            mm16(ps, g, j, o, kt, start=(kt == 0))
                        for kp in range(F):
                            mmdr(ps, g, j, o, kp, stop=(kp == F - 1))
                        finish_tile(g, j, o, ps)
    return nc


def _get_program():
    if "nc" not in _NC_CACHE:
        nc = _build_program()
        nc.finalize()  # runs Bacc.compile(): reg alloc, event-sem wait splitting
        _NC_CACHE["nc"] = nc
    return _NC_CACHE["nc"]


def _prep_x_shard(xs):
    """[T_CORE, K] f32 -> (xq16 [TG, P, K16*TG_W] fp16,
                           xq8  [TG, P, K8*TG_W] e4m3 of 2*x)."""
    x4 = xs.reshape(TG, TG_W, KT, P)  # [g, u, kt, p]
    xt = x4.transpose(0, 3, 2, 1)  # [g, p, kt, u]
    xq16 = (
        np.ascontiguousarray(xt[:, :, :K16]).astype(MM_NP).reshape(TG, P, K16 * TG_W)
    )
    x8f = np.clip(np.ascontiguousarray(xt[:, :, K16:]) * SX8, -240.0, 240.0)
    xq8 = x8f.astype(F8_NP).reshape(TG, P, K8 * TG_W)
    return xq16, xq8


def _prep_in_maps(x, weight, lora_A, lora_B):
    xf = np.ascontiguousarray(x.reshape(T_TOTAL, K))

    # merged-LoRA weight, computed in fp32 on host: W' = W + 2*B@A
    w_merged = weight + SCALING * (lora_B @ lora_A)

    x_shards = [_prep_x_shard(xf[d * T_CORE : (d + 1) * T_CORE]) for d in range(DP)]
    w_shards = []
    for tp in range(TP):
        wT = np.ascontiguousarray(w_merged[tp * O_CORE : (tp + 1) * O_CORE].T)
        wt16 = np.ascontiguousarray(wT[: K16 * P] * SW16).astype(MM_NP)
        wt8 = np.clip(np.ascontiguousarray(wT[K16 * P :]) * SW8, -240.0, 240.0).astype(
            F8_NP
        )
        w_shards.append((wt16, wt8))

    in_maps = []
    for core in range(8):
        d, tp = core // TP, core % TP
        in_maps.append(
            {
                "xq16": x_shards[d][0],
                "xq8": x_shards[d][1],
                "wt16": w_shards[tp][0],
                "wt8": w_shards[tp][1],
            }
        )
    return in_maps


def _gather(results):
    out = np.empty((T_TOTAL, D_OUT), dtype=np.float32)
    for core in range(8):
        d, tp = core // TP, core % TP
        out[d * T_CORE : (d + 1) * T_CORE, tp * O_CORE : (tp + 1) * O_CORE] = results[
            core
        ]["out"]
    return out.reshape(B, S, D_OUT)


def run(x, weight, lora_A, lora_B, trace=False):
    """Returns (output, BassKernelResults)."""
    nc = _get_program()
    in_maps = _prep_in_maps(
        np.asarray(x, dtype=np.float32),
        np.asarray(weight, dtype=np.float32),
        np.asarray(lora_A, dtype=np.float32),
        np.asarray(lora_B, dtype=np.float32),
    )
    res = run_bass_kernel_spmd(nc, in_maps, list(range(8)), trace=trace)
    return _gather(res.results), res


def kernel(x, weight, lora_A, lora_B):
    out, _ = run(x, weight, lora_A, lora_B, trace=False)
    return out


# revision 9
# speedup vs baseline: 1.1720x; 1.0005x over previous
"""Trainium2 Bass kernel for LoraLinear:
    out = x @ W^T + 2.0 * (x @ A^T) @ B^T
    x: [4, 2048, 4096] f32, W: [4096, 4096], A: [64, 4096], B: [4096, 64]

The LoRA update is folded into the weight on the host (merged-LoRA
inference): out = x @ (W + 2*B@A)^T, exactly. The device then runs a pure
[8192 x 4096] @ [4096 x 4096] GEMM.

Sharding across 8 NeuronCores: 4-way data-parallel over tokens x 2-way
tensor-parallel over out-features. Each core computes a [2048 x 2048]
output block. No collectives; the host scatters shards and gathers blocks.

Mixed-precision split-K (the PE at 2.4 GHz is the roofline; fp16 runs
1 row/cycle, fp8e4 with perf_mode=DoubleRow measured at 2 rows/cycle):
  - Most k-blocks run in fp16: x fp16 (near-exact), W' scaled by 2^8
    (exact power-of-2 in fp16).
  - The tail k-blocks run in fp8 e4m3 as DoubleRow pair-matmuls:
    lhsT = x8 [128, 2, 128], rhs = W8 [128, 2, 512], contraction
    256/instr at 2x rate. Scales: x*2, W'*128 -> the product carries
    the same 2^8 factor as the fp16 path, so both accumulate into ONE
    PSUM group; the PSUM->SBUF copy is a DVE tensor_scalar multiply by
    2^-8 (same cost as a plain copy).
  - Error budget (gate 2e-2): measured e4m3 GEMM noise is
    3.18e-2 * sqrt(fp8_fraction_of_K). Output tiles with o < O_SPLIT
    use 6 fp8 pairs, the rest 5 -> fraction 11/32 -> 1.87e-2.
  - Output stores are fp16 (|y| < ~10, quantization 2.4e-4 rms, adds
    nothing in quadrature) to halve the trailing store drain.

Per-core device program (SPMD, same program on all 8 cores):
  - W'^T shards (fp16 22 k-blocks + fp8 12 k-blocks, 14.3 MB) load once
    and stay resident in SBUF.
  - x^T streams once in 8 groups of 256 tokens (fp16 part chunked 2
    k-blocks per DMA; fp8 part 1 pair per DMA).
  - Per 128-token tile and 512-wide out-feature tile: 20-22 fp16
    matmuls + 5-6 DoubleRow fp8 matmuls accumulate into one PSUM bank,
    DVE tensor_scalar copy to SBUF fp16, store alternating between the
    two HWDGE queues so the trailing stores drain 2-wide.
  - Startup: the first group's matmuls run k-OUTER across all 8 PSUM
    banks, consuming W blocks as they arrive from HBM instead of
    stalling until the full weight is resident; the first three fp16
    W blocks are chunked per o-tile so the first matmuls start after
    ~192 KB arrives instead of ~576 KB.
"""

import numpy as np
import ml_dtypes

import concourse.mybir as mybir
import concourse.tile as tile
from concourse import bacc
from concourse.bass_utils import run_bass_kernel_spmd

# problem dims (hardcoded per harness contract)
B, S, D_IN, D_OUT, R = 4, 2048, 4096, 4096, 64
SCALING = 2.0

T_TOTAL = B * S  # 8192 tokens
DP, TP = 4, 2  # token-parallel x feature-parallel over 8 cores
T_CORE = T_TOTAL // DP  # 2048
O_CORE = D_OUT // TP  # 2048
K = D_IN  # 4096

P = 128  # SBUF partitions / matmul contraction tile
KT = K // P  # 32 k-tiles
# Mixed-precision split-K, tuned to the 2e-2 error gate: output tiles with
# o < O_SPLIT run 6 DoubleRow pairs (k-blocks 20..31 fp8), tiles with
# o >= O_SPLIT run 5 (k-blocks 20,21 stay fp16) -> effective fp8 fraction
# 11/32, predicted rel_l2 = 3.18e-2 * sqrt(11/32) = 1.87e-2.
F6, F5 = 6, 5
O_SPLIT = 2
K8 = 2 * F6  # fp8 k-blocks shipped (k-blocks KT-K8 .. KT-1)
K16 = KT - K8  # always-fp16 k-blocks
KD = 2  # dual blocks (20, 21): shipped in BOTH fp16 and fp8
K16S = K16 + KD  # fp16 k-blocks shipped
TG_W = 2 * P  # tokens per x group (2 token tiles)
TG = T_CORE // TG_W  # 8 groups per core
NO = 512  # matmul moving free dim (one PSUM bank of fp32)
OT = O_CORE // NO  # 4 out-feature tiles per core
W_CHUNKED = 3  # startup fp16 W blocks DMA'd per-o-tile for a fast start

# scales: fp16 path x * (2^8 W'); fp8 path (2 x) * (128 W') — both 2^8.
SX8 = 2.0
SW8 = 128.0
SW16 = 256.0
OUT_SCALE = 2.0**-8

MM_DT = mybir.dt.float16
MM_NP = np.float16
F8_DT = mybir.dt.float8e4
F8_NP = ml_dtypes.float8_e4m3
F32 = mybir.dt.float32
OUT_DT = mybir.dt.float16  # output staged/stored fp16 (|y|<~10, eps 2.4e-4)
DR = mybir.MatmulPerfMode.DoubleRow

_NC_CACHE = {}


def _tile_plan(o):
    """(fp16 k-blocks, DoubleRow pair indices) for out-feature tile o."""
    if o < O_SPLIT:
        return range(K16), range(F6)  # blocks 20..31 via pairs 0..5
    return range(K16S), range(1, F6)  # blocks 20,21 fp16; pairs 1..5


def _build_program():
    nc = bacc.Bacc()
    # xq16[g][p][kt*256+u] = fp16 x^T[kt*128+p, g*256+u]        (kt < K16S)
    # xq8 [g][p][kk*256+u] = e4m3 2*x^T[(K16+kk)*128+p, g*256+u] (kk < K8)
    xq16 = nc.declare_dram_parameter(
        "xq16", [TG, P, K16S * TG_W], MM_DT, isOutput=False
    )
    xq8 = nc.declare_dram_parameter("xq8", [TG, P, K8 * TG_W], F8_DT, isOutput=False)
    wt16 = nc.declare_dram_parameter("wt16", [K16S * P, O_CORE], MM_DT, isOutput=False)
    wt8 = nc.declare_dram_parameter("wt8", [K8 * P, O_CORE], F8_DT, isOutput=False)
    out = nc.declare_dram_parameter("out", [T_CORE, O_CORE], OUT_DT, isOutput=True)

    with tile.TileContext(nc) as tc:
        with (
            tc.tile_pool(name="wres16", bufs=1) as wres16,
            tc.tile_pool(name="wres8", bufs=1) as wres8,
            tc.tile_pool(name="xin16", bufs=2) as xin16,
            tc.tile_pool(name="xin8", bufs=2) as xin8,
            tc.tile_pool(name="ostage", bufs=4) as ostage,
            tc.tile_pool(name="psacc", bufs=8, space="PSUM") as psacc,
        ):
            # resident W'^T: fp16 k-blocks side by side, fp8 likewise.
            wtile16 = wres16.tile([P, K16S * O_CORE], MM_DT, name="wtile16")
            wtile8 = wres8.tile([P, K8 * O_CORE], F8_DT, name="wtile8")
            wt16_r = wt16[:].rearrange("(kt p) o -> kt p o", p=P)
            wt8_r = wt8[:].rearrange("(kk p) o -> kk p o", p=P)
            # 3D view for DoubleRow rhs APs [128, 2, free]
            w8_3d = wtile8[:].rearrange("p (kk o) -> p kk o", o=O_CORE)

            x16tiles, x8tiles, x8views = {}, {}, {}

            def w16_dma(eng, kt):
                eng.dma_start(
                    out=wtile16[:, kt * O_CORE : (kt + 1) * O_CORE], in_=wt16_r[kt]
                )

            def w8_dma(eng, kk):
                eng.dma_start(
                    out=wtile8[:, kk * O_CORE : (kk + 1) * O_CORE], in_=wt8_r[kk]
                )

            def load_x(g):
                xt = xin16.tile([P, K16S * TG_W], MM_DT, name="x16t", tag="x16t")
                for c in range(K16S // 2):  # 2 k-blocks per DMA
                    nc.scalar.dma_start(
                        out=xt[:, c * 512 : (c + 1) * 512],
                        in_=xq16[g][:, c * 512 : (c + 1) * 512],
                    )
                x8t = xin8.tile([P, K8 * TG_W], F8_DT, name="x8t", tag="x8t")
                for c in range(F6):  # one DoubleRow pair per DMA
                    nc.scalar.dma_start(
                        out=x8t[:, c * 512 : (c + 1) * 512],
                        in_=xq8[g][:, c * 512 : (c + 1) * 512],
                    )
                x16tiles[g], x8tiles[g] = xt, x8t
                x8views[g] = x8t[:].rearrange("p (kk u) -> p kk u", u=TG_W)

            def mm16(ps, g, j, o, kt, start):
                nc.tensor.matmul(
                    ps[:],
                    x16tiles[g][:, kt * TG_W + j * P : kt * TG_W + (j + 1) * P],
                    wtile16[:, kt * O_CORE + o * NO : kt * O_CORE + o * NO + NO],
                    start=start,
                    stop=False,
                )

            def mmdr(ps, g, j, o, kp, stop):
                nc.tensor.matmul(
                    ps[:],
                    x8views[g][:, 2 * kp : 2 * kp + 2, j * P : (j + 1) * P],
                    w8_3d[:, 2 * kp : 2 * kp + 2, o * NO : (o + 1) * NO],
                    start=False,
                    stop=stop,
                    perf_mode=DR,
                )

            def finish_tile(g, j, o, ps):
                osb = ostage.tile([P, NO], OUT_DT, name="osb")
                nc.vector.tensor_scalar_mul(osb[:], ps[:], OUT_SCALE)
                t = g * 2 + j
                # alternate store queue so the trailing stores drain 2-wide
                eng = nc.sync if (t + o) % 2 == 0 else nc.scalar
                eng.dma_start(
                    out=out[t * P : (t + 1) * P, o * NO : (o + 1) * NO],
                    in_=osb[:],
                )

            # --- startup: consume W blocks AS THEY ARRIVE, k-outer over all
            # 8 PSUM banks so each block gets ~1.7us of PE work vs ~1.4us
            # arrival, instead of stalling until the full W is resident.
            # Both HWDGE queues carry the startup stream in exact consumption
            # order, balanced: per k-block, the x slice then the W block,
            # alternating queues by k parity.
            xt0 = xin16.tile([P, K16S * TG_W], MM_DT, name="x16t", tag="x16t")
            for kt in range(K16S):
                eng = nc.sync if kt % 2 == 0 else nc.scalar
                eng.dma_start(
                    out=xt0[:, kt * TG_W : (kt + 1) * TG_W],
                    in_=xq16[0][:, kt * TG_W : (kt + 1) * TG_W],
                )
                if kt < W_CHUNKED:
                    # early W blocks in o-tile chunks (4 x 128 KB) so the
                    # first matmuls start as soon as ~192 KB has arrived
                    for o in range(OT):
                        e2 = nc.scalar if (kt + o) % 2 == 0 else nc.sync
                        e2.dma_start(
                            out=wtile16[:, kt * O_CORE + o * NO : kt * O_CORE + (o + 1) * NO],
                            in_=wt16_r[kt][:, o * NO : (o + 1) * NO],
                        )
                else:
                    w16_dma(eng, kt)
            x8t0 = xin8.tile([P, K8 * TG_W], F8_DT, name="x8t", tag="x8t")
            for kk in range(K8):
                eng = nc.sync if kk % 2 == 0 else nc.scalar
                eng.dma_start(
                    out=x8t0[:, kk * TG_W : (kk + 1) * TG_W],
                    in_=xq8[0][:, kk * TG_W : (kk + 1) * TG_W],
                )
                w8_dma(eng, kk)
            x16tiles[0], x8tiles[0] = xt0, x8t0
            x8views[0] = x8t0[:].rearrange("p (kk u) -> p kk u", u=TG_W)

            start_ps = {
                (j, o): psacc.tile([P, NO], F32, name="ps", tag="ps")
                for j in range(2)
                for o in range(OT)
            }
            # fp16 k-blocks in arrival order; each only for the tiles that
            # use it in fp16 per the o-split plan.
            for kt in range(K16S):
                for o in range(OT):
                    if kt in _tile_plan(o)[0]:
                        for j in range(2):
                            mm16(start_ps[j, o], 0, j, o, kt, start=(kt == 0))
            for kp in range(F6):
                for o in range(OT):
                    kps = _tile_plan(o)[1]
                    if kp in kps:
                        for j in range(2):
                            mmdr(start_ps[j, o], 0, j, o, kp, stop=(kp == kps[-1]))
            for j in range(2):
                for o in range(OT):
                    finish_tile(0, j, o, start_ps[j, o])

            # --- steady state ---
            for g in range(1, TG):
                load_x(g)
                for j in range(2):
                    for o in range(OT):
                        kts, kps = _tile_plan(o)
                        ps = psacc.tile([P, NO], F32, name="ps", tag="ps")
                        for kt in kts:
                            mm16(ps, g, j, o, kt, start=(kt == 0))
                        for kp in kps:
                            mmdr(ps, g, j, o, kp, stop=(kp == kps[-1]))
                        finish_tile(g, j, o, ps)
    return nc


def _get_program():
    if "nc" not in _NC_CACHE:
        nc = _build_program()
        nc.finalize()  # runs Bacc.compile(): reg alloc, event-sem wait splitting
        _NC_CACHE["nc"] = nc
    return _NC_CACHE["nc"]


def _prep_x_shard(xs):
    """[T_CORE, K] f32 -> (xq16 [TG, P, K16S*TG_W] fp16,
                           xq8  [TG, P, K8*TG_W] e4m3 of 2*x)."""
    x4 = xs.reshape(TG, TG_W, KT, P)  # [g, u, kt, p]
    xt = x4.transpose(0, 3, 2, 1)  # [g, p, kt, u]
    xq16 = (
        np.ascontiguousarray(xt[:, :, :K16S]).astype(MM_NP).reshape(TG, P, K16S * TG_W)
    )
    x8f = np.clip(np.ascontiguousarray(xt[:, :, K16:]) * SX8, -240.0, 240.0)
    xq8 = x8f.astype(F8_NP).reshape(TG, P, K8 * TG_W)
    return xq16, xq8


def _prep_in_maps(x, weight, lora_A, lora_B):
    xf = np.ascontiguousarray(x.reshape(T_TOTAL, K))

    # merged-LoRA weight, computed in fp32 on host: W' = W + 2*B@A
    w_merged = weight + SCALING * (lora_B @ lora_A)

    x_shards = [_prep_x_shard(xf[d * T_CORE : (d + 1) * T_CORE]) for d in range(DP)]
    w_shards = []
    for tp in range(TP):
        wT = np.ascontiguousarray(w_merged[tp * O_CORE : (tp + 1) * O_CORE].T)
        wt16 = np.ascontiguousarray(wT[: K16S * P] * SW16).astype(MM_NP)
        wt8 = np.clip(np.ascontiguousarray(wT[K16 * P :]) * SW8, -240.0, 240.0).astype(
            F8_NP
        )
        w_shards.append((wt16, wt8))

    in_maps = []
    for core in range(8):
        d, tp = core // TP, core % TP
        in_maps.append(
            {
                "xq16": x_shards[d][0],
                "xq8": x_shards[d][1],
                "wt16": w_shards[tp][0],
                "wt8": w_shards[tp][1],
            }
        )
    return in_maps


def _gather(results):
    out = np.empty((T_TOTAL, D_OUT), dtype=np.float32)
    for core in range(8):
        d, tp = core // TP, core % TP
        out[d * T_CORE : (d + 1) * T_CORE, tp * O_CORE : (tp + 1) * O_CORE] = results[
            core
        ]["out"]
    return out.reshape(B, S, D_OUT)


def run(x, weight, lora_A, lora_B, trace=False):
    """Returns (output, BassKernelResults)."""
    nc = _get_program()
    in_maps = _prep_in_maps(
        np.asarray(x, dtype=np.float32),
        np.asarray(weight, dtype=np.float32),
        np.asarray(lora_A, dtype=np.float32),
        np.asarray(lora_B, dtype=np.float32),
    )
    res = run_bass_kernel_spmd(nc, in_maps, list(range(8)), trace=trace)
    return _gather(res.results), res


def kernel(x, weight, lora_A, lora_B):
    out, _ = run(x, weight, lora_A, lora_B, trace=False)
    return out


# revision 13
# speedup vs baseline: 1.1880x; 1.0137x over previous
"""Trainium2 Bass kernel for LoraLinear:
    out = x @ W^T + 2.0 * (x @ A^T) @ B^T
    x: [4, 2048, 4096] f32, W: [4096, 4096], A: [64, 4096], B: [4096, 64]

The LoRA update is folded into the weight on the host (merged-LoRA
inference): out = x @ (W + 2*B@A)^T, exactly. The device then runs a pure
[8192 x 4096] @ [4096 x 4096] GEMM.

Sharding across 8 NeuronCores: 4-way data-parallel over tokens x 2-way
tensor-parallel over out-features. Each core computes a [2048 x 2048]
output block. No collectives; the host scatters shards and gathers blocks.

Mixed-precision split-K (the PE at 2.4 GHz is the roofline; fp16 runs
1 row/cycle, fp8e4 with perf_mode=DoubleRow measured at 2 rows/cycle):
  - Most k-blocks run in fp16: x fp16 (near-exact), W' scaled by 2^8
    (exact power-of-2 in fp16).
  - The tail k-blocks run in fp8 e4m3 as DoubleRow pair-matmuls:
    lhsT = x8 [128, 2, 128], rhs = W8 [128, 2, 512], contraction
    256/instr at 2x rate. Scales: x*2, W'*128 -> the product carries
    the same 2^8 factor as the fp16 path, so both accumulate into ONE
    PSUM group; the PSUM->SBUF copy is a DVE tensor_scalar multiply by
    2^-8 (same cost as a plain copy).
  - Error budget (gate 2e-2): measured e4m3 GEMM noise is
    3.18e-2 * sqrt(fp8_fraction_of_K). Output tiles with o < O_SPLIT
    use 6 fp8 pairs, the rest 5 -> fraction 11/32 -> 1.87e-2.
  - Output stores are fp16 (|y| < ~10, quantization 2.4e-4 rms, adds
    nothing in quadrature) to halve the trailing store drain.

Per-core device program (SPMD, same program on all 8 cores):
  - W'^T shards (fp16 22 k-blocks + fp8 12 k-blocks, 14.3 MB) load once
    and stay resident in SBUF.
  - x^T streams once in 8 groups of 256 tokens (fp16 part chunked 2
    k-blocks per DMA; fp8 part 1 pair per DMA).
  - Per 128-token tile and 512-wide out-feature tile: 20-22 fp16
    matmuls + 5-6 DoubleRow fp8 matmuls accumulate into one PSUM bank,
    DVE tensor_scalar copy to SBUF fp16, store alternating between the
    two HWDGE queues so the trailing stores drain 2-wide.
  - Startup: the first group's matmuls run k-OUTER across all 8 PSUM
    banks, consuming W blocks as they arrive from HBM instead of
    stalling until the full weight is resident; the first three fp16
    W blocks are chunked per o-tile so the first matmuls start after
    ~192 KB arrives instead of ~576 KB.
"""

import numpy as np
import ml_dtypes

import concourse.mybir as mybir
import concourse.tile as tile
from concourse import bacc
from concourse.bass_utils import run_bass_kernel_spmd

# problem dims (hardcoded per harness contract)
B, S, D_IN, D_OUT, R = 4, 2048, 4096, 4096, 64
SCALING = 2.0

T_TOTAL = B * S  # 8192 tokens
DP, TP = 4, 2  # token-parallel x feature-parallel over 8 cores
T_CORE = T_TOTAL // DP  # 2048
O_CORE = D_OUT // TP  # 2048
K = D_IN  # 4096

P = 128  # SBUF partitions / matmul contraction tile
KT = K // P  # 32 k-tiles
# Mixed-precision split-K, tuned to the 2e-2 error gate: output tiles with
# o < O_SPLIT run 6 DoubleRow pairs (k-blocks 20..31 fp8), tiles with
# o >= O_SPLIT run 5 (k-blocks 20,21 stay fp16) -> effective fp8 fraction
# 11/32, predicted rel_l2 = 3.18e-2 * sqrt(11/32) = 1.87e-2.
F6, F5 = 6, 5
O_SPLIT = 2
K8 = 2 * F6  # fp8 k-blocks shipped (k-blocks KT-K8 .. KT-1)
K16 = KT - K8  # always-fp16 k-blocks
KD = 2  # dual blocks (20, 21): shipped in BOTH fp16 and fp8
K16S = K16 + KD  # fp16 k-blocks shipped
TG_W = 2 * P  # tokens per x group (2 token tiles)
TG = T_CORE // TG_W  # 8 groups per core
NO = 512  # matmul moving free dim (one PSUM bank of fp32)
OT = O_CORE // NO  # 4 out-feature tiles per core

# scales: fp16 path x * (2^8 W'); fp8 path (2 x) * (128 W') — both 2^8.
SX8 = 2.0
SW8 = 128.0
SW16 = 256.0
OUT_SCALE = 2.0**-8

MM_DT = mybir.dt.float16
MM_NP = np.float16
F8_DT = mybir.dt.float8e4
F8_NP = ml_dtypes.float8_e4m3
F32 = mybir.dt.float32
OUT_DT = mybir.dt.float16  # output staged/stored fp16 (|y|<~10, eps 2.4e-4)
DR = mybir.MatmulPerfMode.DoubleRow

_NC_CACHE = {}


def _tile_plan(o):
    """(fp16 k-blocks, DoubleRow pair indices) for out-feature tile o."""
    if o < O_SPLIT:
        return range(K16), range(F6)  # blocks 20..31 via pairs 0..5
    return range(K16S), range(1, F6)  # blocks 20,21 fp16; pairs 1..5


def _build_program():
    nc = bacc.Bacc()
    # xq16[g][p][kt*256+u] = fp16 x^T[kt*128+p, g*256+u]        (kt < K16S)
    # xq8 [g][p][kk*256+u] = e4m3 2*x^T[(K16+kk)*128+p, g*256+u] (kk < K8)
    xq16 = nc.declare_dram_parameter(
        "xq16", [TG, P, K16S * TG_W], MM_DT, isOutput=False
    )
    xq8 = nc.declare_dram_parameter("xq8", [TG, P, K8 * TG_W], F8_DT, isOutput=False)
    wt16 = nc.declare_dram_parameter("wt16", [K16S * P, O_CORE], MM_DT, isOutput=False)
    wt8 = nc.declare_dram_parameter("wt8", [K8 * P, O_CORE], F8_DT, isOutput=False)
    out = nc.declare_dram_parameter("out", [T_CORE, O_CORE], OUT_DT, isOutput=True)

    with tile.TileContext(nc) as tc:
        with (
            tc.tile_pool(name="wres16", bufs=1) as wres16,
            tc.tile_pool(name="wres8", bufs=1) as wres8,
            tc.tile_pool(name="xin16", bufs=2) as xin16,
            tc.tile_pool(name="xin8", bufs=2) as xin8,
            tc.tile_pool(name="ostage", bufs=4) as ostage,
            tc.tile_pool(name="psacc", bufs=8, space="PSUM") as psacc,
        ):
            # resident W'^T: fp16 k-blocks side by side, fp8 likewise.
            wtile16 = wres16.tile([P, K16S * O_CORE], MM_DT, name="wtile16")
            wtile8 = wres8.tile([P, K8 * O_CORE], F8_DT, name="wtile8")
            wt16_r = wt16[:].rearrange("(kt p) o -> kt p o", p=P)
            wt8_r = wt8[:].rearrange("(kk p) o -> kk p o", p=P)
            # 3D view for DoubleRow rhs APs [128, 2, free]
            w8_3d = wtile8[:].rearrange("p (kk o) -> p kk o", o=O_CORE)

            x16tiles, x8tiles, x8views = {}, {}, {}

            def w16_dma(eng, kt):
                eng.dma_start(
                    out=wtile16[:, kt * O_CORE : (kt + 1) * O_CORE], in_=wt16_r[kt]
                )

            def w8_dma(eng, kk):
                eng.dma_start(
                    out=wtile8[:, kk * O_CORE : (kk + 1) * O_CORE], in_=wt8_r[kk]
                )

            def load_x(g):
                xt = xin16.tile([P, K16S * TG_W], MM_DT, name="x16t", tag="x16t")
                for c in range(K16S // 2):  # 2 k-blocks per DMA
                    nc.scalar.dma_start(
                        out=xt[:, c * 512 : (c + 1) * 512],
                        in_=xq16[g][:, c * 512 : (c + 1) * 512],
                    )
                x8t = xin8.tile([P, K8 * TG_W], F8_DT, name="x8t", tag="x8t")
                for c in range(F6):  # one DoubleRow pair per DMA
                    nc.scalar.dma_start(
                        out=x8t[:, c * 512 : (c + 1) * 512],
                        in_=xq8[g][:, c * 512 : (c + 1) * 512],
                    )
                x16tiles[g], x8tiles[g] = xt, x8t
                x8views[g] = x8t[:].rearrange("p (kk u) -> p kk u", u=TG_W)

            def mm16_raw(ps, g, j, o, kt, start, stop):
                nc.tensor.matmul(
                    ps[:],
                    x16tiles[g][:, kt * TG_W + j * P : kt * TG_W + (j + 1) * P],
                    wtile16[:, kt * O_CORE + o * NO : kt * O_CORE + o * NO + NO],
                    start=start,
                    stop=stop,
                )

            def mmdr_raw(ps, g, j, o, kp, start, stop):
                nc.tensor.matmul(
                    ps[:],
                    x8views[g][:, 2 * kp : 2 * kp + 2, j * P : (j + 1) * P],
                    w8_3d[:, 2 * kp : 2 * kp + 2, o * NO : (o + 1) * NO],
                    start=start,
                    stop=stop,
                    perf_mode=DR,
                )

            def finish_tile(g, j, o, ps):
                osb = ostage.tile([P, NO], OUT_DT, name="osb")
                nc.vector.tensor_scalar_mul(osb[:], ps[:], OUT_SCALE)
                t = g * 2 + j
                # alternate store queue so the trailing stores drain 2-wide
                eng = nc.sync if (t + o) % 2 == 0 else nc.scalar
                eng.dma_start(
                    out=out[t * P : (t + 1) * P, o * NO : (o + 1) * NO],
                    in_=osb[:],
                )

            # --- startup: consume W blocks AS THEY ARRIVE, k-OUTER over all
            # 8 PSUM banks. The stream is ordered fp8-pairs-first: a 576 KB
            # fp8 pair feeds ~1.7us of PE work (2x the work-per-byte of an
            # fp16 block), which keeps the PE fed while the DMA engines are
            # still ramping. Both HWDGE queues carry the stream in exact
            # consumption order, alternating by block parity. The half-
            # consumed blocks (fp8 pair 0, fp16 blocks 20/21) stream last.
            x8t0 = xin8.tile([P, K8 * TG_W], F8_DT, name="x8t", tag="x8t")
            xt0 = xin16.tile([P, K16S * TG_W], MM_DT, name="x16t", tag="x16t")
            pair_order = [1, 2, 3, 4, 5, 0]
            for i, kp in enumerate(pair_order):
                eng = nc.sync if i % 2 == 0 else nc.scalar
                eng.dma_start(
                    out=x8t0[:, kp * 512 : (kp + 1) * 512],
                    in_=xq8[0][:, kp * 512 : (kp + 1) * 512],
                )
                if i == 0:
                    # first pair in o-tile chunks (8 x 64 KB, both kk of the
                    # pair) so the first matmul starts after ~192 KB arrives
                    for o in range(OT):
                        for kk in (2 * kp, 2 * kp + 1):
                            e2 = nc.scalar if (o + kk) % 2 == 0 else nc.sync
                            e2.dma_start(
                                out=wtile8[
                                    :, kk * O_CORE + o * NO : kk * O_CORE + (o + 1) * NO
                                ],
                                in_=wt8_r[kk][:, o * NO : (o + 1) * NO],
                            )
                else:
                    w8_dma(eng, 2 * kp)
                    w8_dma(eng, 2 * kp + 1)
            for kt in range(K16S):
                eng = nc.sync if kt % 2 == 0 else nc.scalar
                eng.dma_start(
                    out=xt0[:, kt * TG_W : (kt + 1) * TG_W],
                    in_=xq16[0][:, kt * TG_W : (kt + 1) * TG_W],
                )
                w16_dma(eng, kt)
            x16tiles[0], x8tiles[0] = xt0, x8t0
            x8views[0] = x8t0[:].rearrange("p (kk u) -> p kk u", u=TG_W)

            start_ps = {
                (j, o): psacc.tile([P, NO], F32, name="ps", tag="ps")
                for j in range(2)
                for o in range(OT)
            }
            # issue in arrival order; start on the first mm issued per tile,
            # stop on the last.
            started = set()
            remaining = {
                (j, o): len(_tile_plan(o)[0]) + len(_tile_plan(o)[1])
                for j in range(2)
                for o in range(OT)
            }

            def issue_start(kind, kt_or_kp):
                for o in range(OT):
                    kts, kps = _tile_plan(o)
                    use = kt_or_kp in (kps if kind == "dr" else kts)
                    if not use:
                        continue
                    for j in range(2):
                        key = (j, o)
                        first = key not in started
                        started.add(key)
                        remaining[key] -= 1
                        last = remaining[key] == 0
                        if kind == "dr":
                            mmdr_raw(
                                start_ps[j, o], 0, j, o, kt_or_kp, first, last
                            )
                        else:
                            mm16_raw(
                                start_ps[j, o], 0, j, o, kt_or_kp, first, last
                            )

            for kp in pair_order:
                issue_start("dr", kp)
            for kt in range(K16S):
                issue_start("fp16", kt)
            for j in range(2):
                for o in range(OT):
                    finish_tile(0, j, o, start_ps[j, o])

            # --- steady state ---
            for g in range(1, TG):
                load_x(g)
                for j in range(2):
                    for o in range(OT):
                        kts, kps = _tile_plan(o)
                        ps = psacc.tile([P, NO], F32, name="ps", tag="ps")
                        for kt in kts:
                            mm16_raw(ps, g, j, o, kt, kt == 0, False)
                        for kp in kps:
                            mmdr_raw(ps, g, j, o, kp, False, kp == kps[-1])
                        finish_tile(g, j, o, ps)
    return nc


def _get_program():
    if "nc" not in _NC_CACHE:
        nc = _build_program()
        nc.finalize()  # runs Bacc.compile(): reg alloc, event-sem wait splitting
        _NC_CACHE["nc"] = nc
    return _NC_CACHE["nc"]


def _prep_x_shard(xs):
    """[T_CORE, K] f32 -> (xq16 [TG, P, K16S*TG_W] fp16,
                           xq8  [TG, P, K8*TG_W] e4m3 of 2*x)."""
    x4 = xs.reshape(TG, TG_W, KT, P)  # [g, u, kt, p]
    xt = x4.transpose(0, 3, 2, 1)  # [g, p, kt, u]
    xq16 = (
        np.ascontiguousarray(xt[:, :, :K16S]).astype(MM_NP).reshape(TG, P, K16S * TG_W)
    )
    x8f = np.clip(np.ascontiguousarray(xt[:, :, K16:]) * SX8, -240.0, 240.0)
    xq8 = x8f.astype(F8_NP).reshape(TG, P, K8 * TG_W)
    return xq16, xq8


def _prep_in_maps(x, weight, lora_A, lora_B):
    xf = np.ascontiguousarray(x.reshape(T_TOTAL, K))

    # merged-LoRA weight, computed in fp32 on host: W' = W + 2*B@A
    w_merged = weight + SCALING * (lora_B @ lora_A)

    x_shards = [_prep_x_shard(xf[d * T_CORE : (d + 1) * T_CORE]) for d in range(DP)]
    w_shards = []
    for tp in range(TP):
        wT = np.ascontiguousarray(w_merged[tp * O_CORE : (tp + 1) * O_CORE].T)
        wt16 = np.ascontiguousarray(wT[: K16S * P] * SW16).astype(MM_NP)
        wt8 = np.clip(np.ascontiguousarray(wT[K16 * P :]) * SW8, -240.0, 240.0).astype(
            F8_NP
        )
        w_shards.append((wt16, wt8))

    in_maps = []
    for core in range(8):
        d, tp = core // TP, core % TP
        in_maps.append(
            {
                "xq16": x_shards[d][0],
                "xq8": x_shards[d][1],
                "wt16": w_shards[tp][0],
                "wt8": w_shards[tp][1],
            }
        )
    return in_maps


def _gather(results):
    out = np.empty((T_TOTAL, D_OUT), dtype=np.float32)
    for core in range(8):
        d, tp = core // TP, core % TP
        out[d * T_CORE : (d + 1) * T_CORE, tp * O_CORE : (tp + 1) * O_CORE] = results[
            core
        ]["out"]
    return out.reshape(B, S, D_OUT)


def run(x, weight, lora_A, lora_B, trace=False):
    """Returns (output, BassKernelResults)."""
    nc = _get_program()
    in_maps = _prep_in_maps(
        np.asarray(x, dtype=np.float32),
        np.asarray(weight, dtype=np.float32),
        np.asarray(lora_A, dtype=np.float32),
        np.asarray(lora_B, dtype=np.float32),
    )
    res = run_bass_kernel_spmd(nc, in_maps, list(range(8)), trace=trace)
    return _gather(res.results), res


def kernel(x, weight, lora_A, lora_B):
    out, _ = run(x, weight, lora_A, lora_B, trace=False)
    return out
